# revision 8
# baseline (speedup 1.0000x reference)
"""Trainium2 Bass kernel for multi-head attention (dense transformer block).

Reference computation (per batch element):
    qkv = x @ w_qkv                      # [N, 3C]
    q, k, v = split heads (H=12, HD=64)
    out = softmax(q k^T * HD**-0.5) v    # full [N, N] scores
    out = merge_heads(out) @ w_proj + b_proj

Distribution: pure data parallel over the batch dim — B=8 batch elements,
8 NeuronCores, one element per core.  Weights are replicated.  No
collectives; each core computes its full [2048, 768] output.

Per-core design (cost-model-driven; matmul cost = out-free-size rows):
  * xT [768, 2048] bf16 via cast DMA + DMA-xbar transpose (as before).
  * qkT[j, n]: q/k for a head pair packed on 128 partitions (head A rows
    0-63, head B rows 64-127).
  * scoresT per (chunk c of 512 queries, pair, key-tile m): keys on
    partitions, queries free.  [128, 1024] (2 heads x 512 q) per m.
  * exp on TWO engines: ScalarE (ACT) and GpSimd (Pool) both run
    InstActivation(Exp, scale=1/8); tiles alternate 5:3 so neither is a
    bottleneck.  Output at [128, 1024] bf16.
  * attnV uses `at` as the STATIONARY side: lhsT = at[:, 128-query
    slice] (M=128), rhs = [v_h | ones] [128, 65] -> out [128 q, 65]
    where col 64 accumulates the softmax denominator.  8 matmuls of
    N=65 per m-step (8x65=520 rows vs 2048 in the v-stationary form,
    and the ones column makes the separate denominator matmuls free).
  * per (c, pair): two 1-bank PSUM accumulators (head A/B), 4 query
    groups x 65 cols each; after the 16-m sweep: DVE reciprocal of the
    D columns, then 8 tensor_scalar multiplies (per-partition scalar =
    recip) evict normalized [q, feat] bf16 tiles.
  * outQ [q, feat] bounced to DRAM and DMA-xbar transposed into
    outT [768, 2048] (feature-on-partition) for the projection.
  * projection + bias (DVE add) unchanged; proj for chunk c-1 is slotted
    through chunk c's m-stream; the whole attention is one flat
    software-pipelined stream (attnV lags scores by one m-step).
"""

import os

import numpy as np

import concourse.bass as bass
import concourse.mybir as mybir
from concourse import bacc, bass_utils
from concourse.tile import TileContext

F32 = mybir.dt.float32
BF16 = mybir.dt.bfloat16
AF = mybir.ActivationFunctionType
IMM = mybir.ImmediateValue

B, N, C = 8, 2048, 768
H, HD = 12, 64
SCALE = HD ** -0.5  # folded into the exp activation
P = 128
NT = N // P          # 16 key tiles
CT = C // P          # 6 feature tiles
NCHUNK = 4           # query chunks of 512
QW = N // NCHUNK     # 512
PAIRS = 6            # head pairs
VW = HD + 1          # 65: v columns + ones column (denominator)

# Schraudolph exp-on-DVE constants: bitcast_bf16(int16(s*EXP_A + EXP_B))
# ~= exp(s*SCALE) with ~1.8% rms multiplicative error.  The int16 convert
# truncates; EXP_C (tuned offline) absorbs that and centres the
# mantissa-interpolation ripple.
LOG2E = 1.4426950408889634
EXP_A = SCALE * LOG2E * 128.0
EXP_C = 6.75
EXP_B = 127.0 * 128.0 - EXP_C
I16 = mybir.dt.int16
# every DVE_EXP_MOD-th exp tile runs on DVE (0 disables)
DVE_EXP_MOD = 3


def _activation_on(nc, eng, out, in_, func, bias=0.0, scale=1.0):
    """InstActivation emitted on an arbitrary engine (ACT or Pool)."""
    if isinstance(bias, float) and func not in (AF.Copy, AF.Reciprocal):
        bias = nc.const_aps.scalar_like(bias, in_)
    ins = [eng.lower_ap(in_)]
    for arg in (bias, scale, 0.0):
        if isinstance(arg, bass.AP):
            ins.append(eng.lower_ap(arg))
        else:
            ins.append(IMM(dtype=mybir.dt.float32, value=arg))
    return eng.add_instruction(
        mybir.InstActivation(
            name=nc.get_next_instruction_name(),
            func=func,
            ins=ins,
            outs=[eng.lower_ap(out)],
        )
    )


def build_nc() -> bass.Bass:
    nc = bacc.Bacc(None)
    x = nc.declare_dram_parameter("x", [N, C], F32, isOutput=False)
    w_qkv = nc.declare_dram_parameter("w_qkv", [C, 3 * C], F32, isOutput=False)
    w_proj = nc.declare_dram_parameter("w_proj", [C, C], F32, isOutput=False)
    b_proj = nc.declare_dram_parameter("b_proj", [C], F32, isOutput=False)
    out = nc.declare_dram_parameter("out", [N, C], F32, isOutput=True)

    with TileContext(nc) as tc:
        with (
            tc.tile_pool(name="const", bufs=1) as cpool,
            tc.tile_pool(name="dram", bufs=1, space="DRAM") as dpool,
            tc.tile_pool(name="oqdram", bufs=2, space="DRAM") as oqd_pool,
            tc.tile_pool(name="at", bufs=6) as at_pool,
            tc.tile_pool(name="oq", bufs=2) as oq_pool,
            tc.tile_pool(name="recip", bufs=2) as recip_pool,
            tc.tile_pool(name="fin", bufs=2) as fin_pool,
            tc.tile_pool(name="psc", bufs=2, space="PSUM") as psum_sc,
            tc.tile_pool(name="pav", bufs=3, space="PSUM") as psum_av,
            tc.tile_pool(name="pproj", bufs=1, space="PSUM") as psum_proj,
        ):
            # ---- persistent SBUF tensors -------------------------------
            w_qkv_sb = cpool.tile([P, CT, 3 * C], BF16, tag="wqkv")
            wproj_sb = cpool.tile([P, CT, C], BF16, tag="wproj")
            b_bc = cpool.tile([P, C], F32, tag="bias")  # bias bcast to 128 rows
            xT = cpool.tile([P, CT, N], BF16, tag="xT")
            qkT = cpool.tile([P, 12, N], BF16, tag="qkT")  # q pairs 0-5, k 6-11
            vp = cpool.tile([P, NT, H * VW], BF16, tag="vp")  # [v_h | 1] per head
            outT = cpool.tile([P, PAIRS, N], BF16, tag="outT")

            # ---- phase 0: load + cast + transpose ----------------------
            nc.vector.memset(vp[:, :, HD :: VW], 1.0)  # ones cols (denominator)
            nc.gpsimd.dma_start(
                out=w_qkv_sb[:], in_=w_qkv.rearrange("(o p) j -> p o j", p=P)
            )
            nc.gpsimd.dma_start(
                out=wproj_sb[:], in_=w_proj.rearrange("(o p) j -> p o j", p=P)
            )
            nc.sync.dma_start(
                out=b_bc[:], in_=b_proj[None, :].to_broadcast((P, C))
            )
            x_bf = dpool.tile([N, C], BF16)
            for ct in range(CT):
                csl = slice(ct * P, (ct + 1) * P)
                nc.gpsimd.dma_start(out=x_bf[:, csl], in_=x[:, csl])
                nc.sync.dma_start_transpose(xT[:, ct, :], x_bf[:, csl])

            # ---- emit helpers ------------------------------------------
            def emit_qk_group(jt: int, c4: int):
                """qkT[:, jt, c4*512:(c4+1)*512] = (w_qkv col block)^T x^T."""
                ps = psum_sc.tile([P, 1024], F32, tag="sc")
                for ct in range(CT):
                    nc.tensor.matmul(
                        ps[:, 0:QW],
                        lhsT=w_qkv_sb[:, ct, jt * P : (jt + 1) * P],
                        rhs=xT[:, ct, c4 * QW : (c4 + 1) * QW],
                        start=(ct == 0),
                        stop=(ct == CT - 1),
                    )
                nc.vector.tensor_copy(
                    out=qkT[:, jt, c4 * QW : (c4 + 1) * QW], in_=ps[:, 0:QW]
                )

            def emit_v_group(nt: int, p: int):
                """vp[:, nt, pair-p head cols] = x-tile @ w_v (natural layout)."""
                ps = psum_proj.tile([P, 512], F32, tag="proj")
                for ct in range(CT):
                    nc.tensor.matmul(
                        ps[:, 0:P],
                        lhsT=xT[:, ct, nt * P : (nt + 1) * P],
                        rhs=w_qkv_sb[:, ct, 2 * C + p * P : 2 * C + (p + 1) * P],
                        start=(ct == 0),
                        stop=(ct == CT - 1),
                    )
                # scatter the two heads' 64-col halves into the 65-col slots
                nc.vector.tensor_copy(
                    out=vp[:, nt, 2 * p * VW : 2 * p * VW + 2 * VW].rearrange(
                        "p (h w) -> p h w", h=2
                    )[:, :, 0:HD],
                    in_=ps[:, 0:P].rearrange("p (h w) -> p h w", h=2),
                )

            def emit_proj_group(nt: int, eo: int, ew: int):
                """final[nt-tile, eo:eo+ew] = outT^T w_proj + b."""
                ps = psum_proj.tile([P, 512], F32, tag="proj")
                for ct in range(CT):
                    nc.tensor.matmul(
                        ps[:, 0:ew],
                        lhsT=outT[:, ct, nt * P : (nt + 1) * P],
                        rhs=wproj_sb[:, ct, eo : eo + ew],
                        start=(ct == 0),
                        stop=(ct == CT - 1),
                    )
                fs = fin_pool.tile([P, 512], F32, tag="fin")
                nc.vector.tensor_tensor(
                    fs[:, 0:ew], ps[:, 0:ew], b_bc[:, eo : eo + ew],
                    mybir.AluOpType.add,
                )
                nc.sync.dma_start(
                    out=out[nt * P : (nt + 1) * P, eo : eo + ew], in_=fs[:, 0:ew]
                )

            def emit_proj_slot(c_done: int, slot: int):
                nt = c_done * 4 + slot // 2
                eo, ew = ((0, 512), (512, 256))[slot % 2]
                emit_proj_group(nt, eo, ew)

            # ---- phase 1 upfront: pair-0 kT + qT(0, chunk0) -------------
            for c4 in range(NCHUNK):
                emit_qk_group(6, c4)
            emit_qk_group(0, 0)

            # ---- phase 2: flat software-pipelined attention stream ------
            # per (c, p): m-sweep over 16 key tiles; attnV lags scores by
            # one step so the PE never waits on the exp engines.
            kt_slots = {1: 0, 4: 1, 7: 2, 10: 3}  # m -> c4 of kT(p+1)
            proj_slots = {  # (p, m) -> slot
                (1, 3): 0, (1, 11): 1, (2, 3): 2, (2, 11): 3,
                (3, 3): 4, (3, 11): 5, (4, 3): 6, (4, 11): 7,
            }

            state = {"i": 0}  # exp tile counter for engine assignment

            def emit_scores(c, p, m):
                qsl = slice(c * QW, (c + 1) * QW)
                msl = slice(m * P, (m + 1) * P)
                sc = psum_sc.tile([P, 1024], F32, tag="sc")
                nc.tensor.matmul(
                    sc[:, 0:QW],
                    lhsT=qkT[0:64, 6 + p, msl],
                    rhs=qkT[0:64, p, qsl],
                    start=True,
                    stop=True,
                )
                nc.tensor.matmul(
                    sc[:, QW : 2 * QW],
                    lhsT=qkT[64:128, 6 + p, msl],
                    rhs=qkT[64:128, p, qsl],
                    start=True,
                    stop=True,
                )
                at = at_pool.tile([P, 1024], BF16, tag="at")
                i = state["i"]
                state["i"] = i + 1
                if DVE_EXP_MOD and i % DVE_EXP_MOD == DVE_EXP_MOD - 1:
                    # approximate exp on the Vector engine (Schraudolph)
                    nc.vector.tensor_scalar(
                        out=at[:].bitcast(I16),
                        in0=sc[:],
                        scalar1=EXP_A,
                        scalar2=EXP_B,
                        op0=mybir.AluOpType.mult,
                        op1=mybir.AluOpType.add,
                    )
                else:
                    _activation_on(nc, nc.scalar, at[:], sc[:], AF.Exp, scale=SCALE)
                return at

            def emit_attnv(av_banks, at, p, m):
                for hh in range(2):
                    h = 2 * p + hh
                    for g in range(4):
                        # start zeroes the whole 2KB zero region (the bank),
                        # so only the first col-group starts and only the
                        # last one stops the accumulation group
                        nc.tensor.matmul(
                            av_banks[hh][:, g * VW : (g + 1) * VW],
                            lhsT=at[:, hh * QW + g * P : hh * QW + (g + 1) * P],
                            rhs=vp[:, m, h * VW : (h + 1) * VW],
                            start=(m == 0 and g == 0),
                            stop=(m == NT - 1 and g == 3),
                        )

            def emit_evict(av_banks, c, p):
                """normalize by the accumulated denominators and store outQ,
                then bounce to DRAM and DMA-transpose into outT."""
                rc = recip_pool.tile([P, 8], F32, tag="recip")
                for hh in range(2):
                    nc.vector.reciprocal(
                        rc[:, 4 * hh : 4 * hh + 4],
                        av_banks[hh][:, HD : 4 * VW : VW],
                    )
                oq = oq_pool.tile([P, 4, P], BF16, tag="oq")
                for hh in range(2):
                    for g in range(4):
                        nc.vector.tensor_scalar(
                            out=oq[:, g, hh * HD : (hh + 1) * HD],
                            in0=av_banks[hh][:, g * VW : g * VW + HD],
                            scalar1=rc[:, 4 * hh + g : 4 * hh + g + 1],
                            scalar2=None,
                            op0=mybir.AluOpType.mult,
                        )
                oqd = oqd_pool.tile([QW, P], BF16)
                nc.sync.dma_start(
                    out=oqd.rearrange("(a p) f -> p a f", p=P), in_=oq[:]
                )
                nc.sync.dma_start_transpose(
                    outT[:, p, c * QW : (c + 1) * QW], oqd[:]
                )

            steps = [
                (c, p, m)
                for c in range(NCHUNK)
                for p in range(PAIRS)
                for m in range(NT)
            ]
            prev = None  # (av_banks, at, c, p, m)
            av_banks = None
            for (c, p, m) in steps:
                if m == 0:
                    av_banks = [
                        psum_av.tile([P, 512], F32, tag="av", name=f"av{c}_{p}_{hh}")
                        for hh in range(2)
                    ]
                # ---- JIT slots ----
                if c == 0:
                    emit_v_group(m, p)
                    if p < PAIRS - 1:
                        if m in kt_slots:
                            emit_qk_group(6 + p + 1, kt_slots[m])
                        elif m == 13:
                            emit_qk_group(p + 1, 0)
                if m == 8 and c < NCHUNK - 1:
                    emit_qk_group(p, c + 1)  # next chunk's qT for this pair
                if c > 0 and (p, m) in proj_slots:
                    emit_proj_slot(c - 1, proj_slots[(p, m)])
                # ---- scores + exp for this step ----
                at = emit_scores(c, p, m)
                cur = (av_banks, at, c, p, m)
                # ---- attnV for the previous step (one-step lag) ----
                if prev is not None:
                    pav, pat, pc, pp, pm = prev
                    emit_attnv(pav, pat, pp, pm)
                    if pm == NT - 1:
                        emit_evict(pav, pc, pp)
                prev = cur
            pav, pat, pc, pp, pm = prev
            emit_attnv(pav, pat, pp, pm)
            emit_evict(pav, pc, pp)
            # tail: proj for the last chunk
            for slot in range(8):
                emit_proj_slot(NCHUNK - 1, slot)

    nc.compile()
    return nc


_NC_CACHE: list = []


def _get_nc() -> bass.Bass:
    if not _NC_CACHE:
        _NC_CACHE.append(build_nc())
    return _NC_CACHE[0]


def run(inputs: dict, trace: bool = False):
    """Run on 8 NeuronCores.  Returns (out [B,N,C] f32, exec_time_ns|None)."""
    nc = _get_nc()
    x = np.ascontiguousarray(np.asarray(inputs["x"], dtype=np.float32))
    w_qkv = np.ascontiguousarray(np.asarray(inputs["w_qkv"], dtype=np.float32))
    w_proj = np.ascontiguousarray(np.asarray(inputs["w_proj"], dtype=np.float32))
    b_proj = np.ascontiguousarray(np.asarray(inputs["b_proj"], dtype=np.float32))
    in_maps = [
        {"x": x[i], "w_qkv": w_qkv, "w_proj": w_proj, "b_proj": b_proj}
        for i in range(B)
    ]
    try:
        res = bass_utils.run_bass_kernel_spmd(
            nc, in_maps, core_ids=list(range(B)), trace=trace
        )
    except ModuleNotFoundError:
        res = bass_utils.run_bass_kernel_spmd(
            nc, in_maps, core_ids=list(range(B)), trace=False
        )
    out = np.stack([res.results[i]["out"] for i in range(B)], axis=0)
    return out.astype(np.float32), res.exec_time_ns


def kernel(x, w_qkv, w_proj, b_proj):
    trace = os.environ.get("BASS_KERNEL_TRACE", "0") == "1"
    out, _ = run(
        {"x": x, "w_qkv": w_qkv, "w_proj": w_proj, "b_proj": b_proj}, trace=trace
    )
    return out


# revision 15
# speedup vs baseline: 1.0395x; 1.0395x over previous
"""Trainium2 Bass kernel for multi-head attention (dense transformer block).

Reference computation (per batch element):
    qkv = x @ w_qkv                      # [N, 3C]
    q, k, v = split heads (H=12, HD=64)
    out = softmax(q k^T * HD**-0.5) v    # full [N, N] scores
    out = merge_heads(out) @ w_proj + b_proj

Distribution: pure data parallel over the batch dim — B=8 batch elements,
8 NeuronCores, one element per core.  Weights are replicated.  No
collectives; each core computes its full [2048, 768] output.

Per-core design (cost-model-driven; matmul cost = out-free-size rows):
  * xT [768, 2048] bf16 via cast DMA + DMA-xbar transpose (as before).
  * qkT[j, n]: q/k for a head pair packed on 128 partitions (head A rows
    0-63, head B rows 64-127).
  * scoresT per (chunk c of 512 queries, pair, key-tile m): keys on
    partitions, queries free.  [128, 1024] (2 heads x 512 q) per m.
  * exp on TWO engines: ScalarE (ACT) and GpSimd (Pool) both run
    InstActivation(Exp, scale=1/8); tiles alternate 5:3 so neither is a
    bottleneck.  Output at [128, 1024] bf16.
  * attnV uses `at` as the STATIONARY side: lhsT = at[:, 128-query
    slice] (M=128), rhs = [v_h | ones] [128, 65] -> out [128 q, 65]
    where col 64 accumulates the softmax denominator.  8 matmuls of
    N=65 per m-step (8x65=520 rows vs 2048 in the v-stationary form,
    and the ones column makes the separate denominator matmuls free).
  * per (c, pair): two 1-bank PSUM accumulators (head A/B), 4 query
    groups x 65 cols each; after the 16-m sweep: DVE reciprocal of the
    D columns, then 8 tensor_scalar multiplies (per-partition scalar =
    recip) evict normalized [q, feat] bf16 tiles.
  * outQ [q, feat] bounced to DRAM and DMA-xbar transposed into
    outT [768, 2048] (feature-on-partition) for the projection.
  * projection + bias (DVE add) unchanged; proj for chunk c-1 is slotted
    through chunk c's m-stream; the whole attention is one flat
    software-pipelined stream (attnV lags scores by one m-step).
"""

import os

import numpy as np

import concourse.bass as bass
import concourse.mybir as mybir
from concourse import bacc, bass_utils
from concourse.tile import TileContext

F32 = mybir.dt.float32
BF16 = mybir.dt.bfloat16
AF = mybir.ActivationFunctionType
IMM = mybir.ImmediateValue

B, N, C = 8, 2048, 768
H, HD = 12, 64
SCALE = HD ** -0.5  # folded into the exp activation
P = 128
NT = N // P          # 16 key tiles
CT = C // P          # 6 feature tiles
NCHUNK = 4           # query chunks of 512
QW = N // NCHUNK     # 512
PAIRS = 6            # head pairs
VW = HD + 1          # 65: v columns + ones column (denominator)

# Schraudolph exp-on-DVE constants: bitcast_bf16(int16(s*EXP_A + EXP_B))
# ~= exp(s*SCALE) with ~1.8% rms multiplicative error.  The int16 convert
# truncates; EXP_C (tuned offline) absorbs that and centres the
# mantissa-interpolation ripple.
LOG2E = 1.4426950408889634
EXP_A = SCALE * LOG2E * 128.0
EXP_C = 6.75
EXP_B = 127.0 * 128.0 - EXP_C
I16 = mybir.dt.int16
# every DVE_EXP_MOD-th exp tile runs on DVE (0 disables)
DVE_EXP_MOD = 3


def _activation_on(nc, eng, out, in_, func, bias=0.0, scale=1.0):
    """InstActivation emitted on an arbitrary engine (ACT or Pool)."""
    if isinstance(bias, float) and func not in (AF.Copy, AF.Reciprocal):
        bias = nc.const_aps.scalar_like(bias, in_)
    ins = [eng.lower_ap(in_)]
    for arg in (bias, scale, 0.0):
        if isinstance(arg, bass.AP):
            ins.append(eng.lower_ap(arg))
        else:
            ins.append(IMM(dtype=mybir.dt.float32, value=arg))
    return eng.add_instruction(
        mybir.InstActivation(
            name=nc.get_next_instruction_name(),
            func=func,
            ins=ins,
            outs=[eng.lower_ap(out)],
        )
    )


def build_nc() -> bass.Bass:
    nc = bacc.Bacc(None)
    x = nc.declare_dram_parameter("x", [N, C], F32, isOutput=False)
    w_qkv = nc.declare_dram_parameter("w_qkv", [C, 3 * C], F32, isOutput=False)
    w_proj = nc.declare_dram_parameter("w_proj", [C, C], F32, isOutput=False)
    b_proj = nc.declare_dram_parameter("b_proj", [C], F32, isOutput=False)
    out = nc.declare_dram_parameter("out", [N, C], F32, isOutput=True)

    with TileContext(nc) as tc:
        with (
            tc.tile_pool(name="const", bufs=1) as cpool,
            tc.tile_pool(name="dram", bufs=1, space="DRAM") as dpool,
            tc.tile_pool(name="oqdram", bufs=2, space="DRAM") as oqd_pool,
            tc.tile_pool(name="at", bufs=6) as at_pool,
            tc.tile_pool(name="oq", bufs=2) as oq_pool,
            tc.tile_pool(name="recip", bufs=2) as recip_pool,
            tc.tile_pool(name="fin", bufs=2) as fin_pool,
            tc.tile_pool(name="psc", bufs=2, space="PSUM") as psum_sc,
            tc.tile_pool(name="pav", bufs=3, space="PSUM") as psum_av,
            tc.tile_pool(name="pproj", bufs=1, space="PSUM") as psum_proj,
        ):
            # ---- persistent SBUF tensors -------------------------------
            w_qkv_sb = cpool.tile([P, CT, 3 * C], BF16, tag="wqkv")
            wproj_sb = cpool.tile([P, CT, C], BF16, tag="wproj")
            b_bc = cpool.tile([P, C], F32, tag="bias")  # bias bcast to 128 rows
            xT = cpool.tile([P, CT, N], BF16, tag="xT")
            qkT = cpool.tile([P, 12, N], BF16, tag="qkT")  # q pairs 0-5, k 6-11
            vp = cpool.tile([P, NT, H * VW], BF16, tag="vp")  # [v_h | 1] per head
            outT = cpool.tile([P, PAIRS, N], BF16, tag="outT")

            # ---- phase 0: load + cast + transpose ----------------------
            # SWDGE (gpsimd) queue order matters: x casts first so the xT
            # transposes (and with them the first qk groups) start ASAP;
            # w_qkv per-ct so accumulation can begin before the full load;
            # w_proj/bias last (first needed at chunk 1).
            nc.vector.memset(vp[:, :, HD :: VW], 1.0)  # ones cols (denominator)
            x_bf = dpool.tile([N, C], BF16)
            for ct in range(CT):
                csl = slice(ct * P, (ct + 1) * P)
                nc.gpsimd.dma_start(out=x_bf[:, csl], in_=x[:, csl])
                nc.sync.dma_start_transpose(xT[:, ct, :], x_bf[:, csl])
            for ct in range(CT):
                nc.gpsimd.dma_start(
                    out=w_qkv_sb[:, ct, :], in_=w_qkv[ct * P : (ct + 1) * P, :]
                )
            nc.gpsimd.dma_start(
                out=wproj_sb[:], in_=w_proj.rearrange("(o p) j -> p o j", p=P)
            )
            nc.sync.dma_start(
                out=b_bc[:], in_=b_proj[None, :].to_broadcast((P, C))
            )

            # ---- emit helpers ------------------------------------------
            def emit_qk_group(jt: int, c4: int):
                """qkT[:, jt, c4*512:(c4+1)*512] = (w_qkv col block)^T x^T."""
                ps = psum_sc.tile([P, 1024], F32, tag="sc")
                for ct in range(CT):
                    nc.tensor.matmul(
                        ps[:, 0:QW],
                        lhsT=w_qkv_sb[:, ct, jt * P : (jt + 1) * P],
                        rhs=xT[:, ct, c4 * QW : (c4 + 1) * QW],
                        start=(ct == 0),
                        stop=(ct == CT - 1),
                    )
                # eviction on ACT (Pool cannot read PSUM on TRN2): keeps
                # DVE free for exp tiles
                nc.scalar.copy(
                    out=qkT[:, jt, c4 * QW : (c4 + 1) * QW], in_=ps[:, 0:QW]
                )

            def emit_v_group(nt: int, p: int):
                """vp[:, nt, pair-p head cols] = x-tile @ w_v (natural layout)."""
                ps = psum_proj.tile([P, 512], F32, tag="proj")
                for ct in range(CT):
                    nc.tensor.matmul(
                        ps[:, 0:P],
                        lhsT=xT[:, ct, nt * P : (nt + 1) * P],
                        rhs=w_qkv_sb[:, ct, 2 * C + p * P : 2 * C + (p + 1) * P],
                        start=(ct == 0),
                        stop=(ct == CT - 1),
                    )
                # scatter the two heads' 64-col halves into the 65-col slots
                nc.vector.tensor_copy(
                    out=vp[:, nt, 2 * p * VW : 2 * p * VW + 2 * VW].rearrange(
                        "p (h w) -> p h w", h=2
                    )[:, :, 0:HD],
                    in_=ps[:, 0:P].rearrange("p (h w) -> p h w", h=2),
                )

            def emit_proj_group(nt: int, eo: int, ew: int):
                """final[nt-tile, eo:eo+ew] = outT^T w_proj + b."""
                ps = psum_proj.tile([P, 512], F32, tag="proj")
                for ct in range(CT):
                    nc.tensor.matmul(
                        ps[:, 0:ew],
                        lhsT=outT[:, ct, nt * P : (nt + 1) * P],
                        rhs=wproj_sb[:, ct, eo : eo + ew],
                        start=(ct == 0),
                        stop=(ct == CT - 1),
                    )
                fs = fin_pool.tile([P, 512], F32, tag="fin")
                nc.vector.tensor_tensor(
                    fs[:, 0:ew], ps[:, 0:ew], b_bc[:, eo : eo + ew],
                    mybir.AluOpType.add,
                )
                nc.sync.dma_start(
                    out=out[nt * P : (nt + 1) * P, eo : eo + ew], in_=fs[:, 0:ew]
                )

            def emit_proj_slot(c_done: int, slot: int):
                nt = c_done * 4 + slot // 2
                eo, ew = ((0, 512), (512, 256))[slot % 2]
                emit_proj_group(nt, eo, ew)

            # ---- phase 1 upfront: pair-0 kT + qT(0, chunk0) -------------
            for c4 in range(NCHUNK):
                emit_qk_group(6, c4)
            emit_qk_group(0, 0)

            # ---- phase 2: flat software-pipelined attention stream ------
            # per (c, p): m-sweep over 16 key tiles; attnV lags scores by
            # one step so the PE never waits on the exp engines.
            kt_slots = {1: 0, 4: 1, 7: 2, 10: 3}  # m -> c4 of kT(p+1)
            proj_slots = {  # (p, m) -> slot
                (1, 3): 0, (1, 11): 1, (2, 3): 2, (2, 11): 3,
                (3, 3): 4, (3, 11): 5, (4, 3): 6, (4, 11): 7,
            }

            state = {"i": 0}  # exp tile counter for engine assignment

            def emit_scores(c, p, m):
                qsl = slice(c * QW, (c + 1) * QW)
                msl = slice(m * P, (m + 1) * P)
                sc = psum_sc.tile([P, 1024], F32, tag="sc")
                nc.tensor.matmul(
                    sc[:, 0:QW],
                    lhsT=qkT[0:64, 6 + p, msl],
                    rhs=qkT[0:64, p, qsl],
                    start=True,
                    stop=True,
                )
                nc.tensor.matmul(
                    sc[:, QW : 2 * QW],
                    lhsT=qkT[64:128, 6 + p, msl],
                    rhs=qkT[64:128, p, qsl],
                    start=True,
                    stop=True,
                )
                at = at_pool.tile([P, 1024], BF16, tag="at")
                i = state["i"]
                state["i"] = i + 1
                if DVE_EXP_MOD and i % DVE_EXP_MOD == DVE_EXP_MOD - 1:
                    # approximate exp on the Vector engine (Schraudolph)
                    nc.vector.tensor_scalar(
                        out=at[:].bitcast(I16),
                        in0=sc[:],
                        scalar1=EXP_A,
                        scalar2=EXP_B,
                        op0=mybir.AluOpType.mult,
                        op1=mybir.AluOpType.add,
                    )
                else:
                    _activation_on(nc, nc.scalar, at[:], sc[:], AF.Exp, scale=SCALE)
                return at

            def emit_attnv(av_banks, at, p, m):
                for hh in range(2):
                    h = 2 * p + hh
                    for g in range(4):
                        # start zeroes the whole 2KB zero region (the bank),
                        # so only the first col-group starts and only the
                        # last one stops the accumulation group
                        nc.tensor.matmul(
                            av_banks[hh][:, g * VW : (g + 1) * VW],
                            lhsT=at[:, hh * QW + g * P : hh * QW + (g + 1) * P],
                            rhs=vp[:, m, h * VW : (h + 1) * VW],
                            start=(m == 0 and g == 0),
                            stop=(m == NT - 1 and g == 3),
                        )

            def emit_evict(av_banks, c, p):
                """normalize by the accumulated denominators and store outQ,
                then bounce to DRAM and DMA-transpose into outT."""
                rc = recip_pool.tile([P, 8], F32, tag="recip")
                for hh in range(2):
                    nc.vector.reciprocal(
                        rc[:, 4 * hh : 4 * hh + 4],
                        av_banks[hh][:, HD : 4 * VW : VW],
                    )
                oq = oq_pool.tile([P, 4, P], BF16, tag="oq")
                for hh in range(2):
                    for g in range(4):
                        nc.vector.tensor_scalar(
                            out=oq[:, g, hh * HD : (hh + 1) * HD],
                            in0=av_banks[hh][:, g * VW : g * VW + HD],
                            scalar1=rc[:, 4 * hh + g : 4 * hh + g + 1],
                            scalar2=None,
                            op0=mybir.AluOpType.mult,
                        )
                oqd = oqd_pool.tile([QW, P], BF16)
                nc.sync.dma_start(
                    out=oqd.rearrange("(a p) f -> p a f", p=P), in_=oq[:]
                )
                nc.sync.dma_start_transpose(
                    outT[:, p, c * QW : (c + 1) * QW], oqd[:]
                )

            steps = [
                (c, p, m)
                for c in range(NCHUNK)
                for p in range(PAIRS)
                for m in range(NT)
            ]
            # attnV trails scores by LAG m-steps so the exp engines have
            # ~LAG full steps of latency headroom before the PE needs `at`
            LAG = 2
            pending = []  # (av_banks, at, c, p, m)
            av_banks = None

            def retire_one():
                pav, pat, pc, pp, pm = pending.pop(0)
                emit_attnv(pav, pat, pp, pm)
                if pm == NT - 1:
                    emit_evict(pav, pc, pp)

            for (c, p, m) in steps:
                if m == 0:
                    av_banks = [
                        psum_av.tile([P, 512], F32, tag="av", name=f"av{c}_{p}_{hh}")
                        for hh in range(2)
                    ]
                # ---- scores + exp for this step ----
                at = emit_scores(c, p, m)
                pending.append((av_banks, at, c, p, m))
                if len(pending) > LAG:
                    retire_one()
                # ---- JIT slots (after the latency-critical work) ----
                if c == 0:
                    emit_v_group(m, p)
                    if p < PAIRS - 1:
                        if m in kt_slots:
                            emit_qk_group(6 + p + 1, kt_slots[m])
                        elif m == 13:
                            emit_qk_group(p + 1, 0)
                if m == 8 and c < NCHUNK - 1:
                    emit_qk_group(p, c + 1)  # next chunk's qT for this pair
                if c > 0 and (p, m) in proj_slots:
                    emit_proj_slot(c - 1, proj_slots[(p, m)])
            while pending:
                retire_one()
            # tail: proj for the last chunk
            for slot in range(8):
                emit_proj_slot(NCHUNK - 1, slot)

    nc.compile()
    return nc


_NC_CACHE: list = []


def _get_nc() -> bass.Bass:
    if not _NC_CACHE:
        _NC_CACHE.append(build_nc())
    return _NC_CACHE[0]


def run(inputs: dict, trace: bool = False):
    """Run on 8 NeuronCores.  Returns (out [B,N,C] f32, exec_time_ns|None)."""
    nc = _get_nc()
    x = np.ascontiguousarray(np.asarray(inputs["x"], dtype=np.float32))
    w_qkv = np.ascontiguousarray(np.asarray(inputs["w_qkv"], dtype=np.float32))
    w_proj = np.ascontiguousarray(np.asarray(inputs["w_proj"], dtype=np.float32))
    b_proj = np.ascontiguousarray(np.asarray(inputs["b_proj"], dtype=np.float32))
    in_maps = [
        {"x": x[i], "w_qkv": w_qkv, "w_proj": w_proj, "b_proj": b_proj}
        for i in range(B)
    ]
    try:
        res = bass_utils.run_bass_kernel_spmd(
            nc, in_maps, core_ids=list(range(B)), trace=trace
        )
    except ModuleNotFoundError:
        res = bass_utils.run_bass_kernel_spmd(
            nc, in_maps, core_ids=list(range(B)), trace=False
        )
    out = np.stack([res.results[i]["out"] for i in range(B)], axis=0)
    return out.astype(np.float32), res.exec_time_ns


def kernel(x, w_qkv, w_proj, b_proj):
    trace = os.environ.get("BASS_KERNEL_TRACE", "0") == "1"
    out, _ = run(
        {"x": x, "w_qkv": w_qkv, "w_proj": w_proj, "b_proj": b_proj}, trace=trace
    )
    return out


# revision 35
# speedup vs baseline: 1.1063x; 1.0643x over previous
"""Trainium2 Bass kernel for multi-head attention (dense transformer block).

Reference computation (per batch element):
    qkv = x @ w_qkv                      # [N, 3C]
    q, k, v = split heads (H=12, HD=64)
    out = softmax(q k^T * HD**-0.5) v    # full [N, N] scores
    out = merge_heads(out) @ w_proj + b_proj

Distribution: pure data parallel over the batch dim — B=8 batch elements,
8 NeuronCores, one element per core.  Weights are replicated.  No
collectives; each core computes its full [2048, 768] output.

Per-core design (cost-model-driven; matmul cost = out-free-size rows):
  * xT [768, 2048] bf16 via cast DMA + DMA-xbar transpose; all phase-0
    DMAs are need-ordered (DMA transfers serialize on one device).
  * qkT[j, n]: q/k for a head pair packed on 128 partitions (head A rows
    0-63, head B rows 64-127).
  * scoresT per (chunk c of 512 queries, pair, key-tile m): keys on
    partitions, queries free.  [128, 1024] (2 heads x 512 q) per m.
  * exp: 2/3 of tiles run exact Exp on ScalarE; every 3rd tile runs a
    Schraudolph approximation on the Vector engine (one tensor_scalar:
    bitcast_bf16(int16(s*A + B)) ~ exp(s/8), ~1.8% rms), keeping either
    engine's throughput and latency below the PE's m-step time.
  * attnV uses `at` as the STATIONARY side: lhsT = at[:, 128-query
    slice] (M=128), rhs = [v_h | ones] [128, 65] -> out [128 q, 65]
    where col 64 accumulates the softmax denominator.  8 matmuls of
    N=65 per m-step (8x65=520 rows vs 2048 in the v-stationary form,
    and the ones column makes the separate denominator matmuls free).
  * per (c, pair): two 1-bank PSUM accumulators (head A/B), 4 query
    groups x 65 cols each; after the 16-m sweep: DVE reciprocal of the
    D columns, then one scalar_tensor_tensor per bank (stride-0
    broadcast recip) evicts normalized [q, feat] bf16 tiles.
  * outQ [q, feat] bounced to DRAM and DMA-xbar transposed into
    outT [768, 2048] (feature-on-partition) for the projection.
  * projection + bias (DVE add): proj for chunk c-1 is slotted through
    chunk c's m-stream; qT prefetch groups borrow the av pool's spare
    psum bank so the scores double-buffer parity is never broken.
  * the whole attention is one flat software-pipelined stream: attnV
    trails scores by LAG=7 m-steps so PE never waits on the exp engines.
"""

import os

import numpy as np

import concourse.bass as bass
import concourse.mybir as mybir
from concourse import bacc, bass_utils
from concourse.tile import TileContext

F32 = mybir.dt.float32
BF16 = mybir.dt.bfloat16
AF = mybir.ActivationFunctionType
IMM = mybir.ImmediateValue

B, N, C = 8, 2048, 768
H, HD = 12, 64
SCALE = HD ** -0.5  # folded into the exp activation
P = 128
NT = N // P          # 16 key tiles
CT = C // P          # 6 feature tiles
NCHUNK = 4           # query chunks of 512
QW = N // NCHUNK     # 512
PAIRS = 6            # head pairs
VW = HD + 1          # 65: v columns + ones column (denominator)

# Schraudolph exp-on-DVE constants: bitcast_bf16(int16(s*EXP_A + EXP_B))
# ~= exp(s*SCALE) with ~1.8% rms multiplicative error.  The int16 convert
# truncates; EXP_C (tuned offline) absorbs that and centres the
# mantissa-interpolation ripple.
LOG2E = 1.4426950408889634
EXP_A = SCALE * LOG2E * 128.0
EXP_C = 6.75
EXP_B = 127.0 * 128.0 - EXP_C
I16 = mybir.dt.int16
# every DVE_EXP_MOD-th exp tile runs on DVE (0 disables)
DVE_EXP_MOD = 3


def _activation_on(nc, eng, out, in_, func, bias=0.0, scale=1.0):
    """InstActivation emitted on an arbitrary engine (ACT or Pool)."""
    if isinstance(bias, float) and func not in (AF.Copy, AF.Reciprocal):
        bias = nc.const_aps.scalar_like(bias, in_)
    ins = [eng.lower_ap(in_)]
    for arg in (bias, scale, 0.0):
        if isinstance(arg, bass.AP):
            ins.append(eng.lower_ap(arg))
        else:
            ins.append(IMM(dtype=mybir.dt.float32, value=arg))
    return eng.add_instruction(
        mybir.InstActivation(
            name=nc.get_next_instruction_name(),
            func=func,
            ins=ins,
            outs=[eng.lower_ap(out)],
        )
    )


def build_nc() -> bass.Bass:
    nc = bacc.Bacc(None)
    x = nc.declare_dram_parameter("x", [N, C], F32, isOutput=False)
    w_qkv = nc.declare_dram_parameter("w_qkv", [C, 3 * C], F32, isOutput=False)
    w_proj = nc.declare_dram_parameter("w_proj", [C, C], F32, isOutput=False)
    b_proj = nc.declare_dram_parameter("b_proj", [C], F32, isOutput=False)
    out = nc.declare_dram_parameter("out", [N, C], F32, isOutput=True)

    with TileContext(nc) as tc:
        with (
            tc.tile_pool(name="const", bufs=1) as cpool,
            tc.tile_pool(name="dram", bufs=1, space="DRAM") as dpool,
            tc.tile_pool(name="oqdram", bufs=2, space="DRAM") as oqd_pool,
            tc.tile_pool(name="at", bufs=9) as at_pool,
            tc.tile_pool(name="oq", bufs=2) as oq_pool,
            tc.tile_pool(name="recip", bufs=2) as recip_pool,
            tc.tile_pool(name="fin", bufs=2) as fin_pool,
            tc.tile_pool(name="psc", bufs=2, space="PSUM") as psum_sc,
            tc.tile_pool(name="pav", bufs=3, space="PSUM") as psum_av,
            tc.tile_pool(name="pproj", bufs=1, space="PSUM") as psum_proj,
        ):
            # ---- persistent SBUF tensors -------------------------------
            w_qkv_sb = cpool.tile([P, CT, 3 * C], BF16, tag="wqkv")
            wproj_sb = cpool.tile([P, CT, C], BF16, tag="wproj")
            b_bc = cpool.tile([P, C], F32, tag="bias")  # bias bcast to 128 rows
            xT = cpool.tile([P, CT, N], BF16, tag="xT")
            qkT = cpool.tile([P, 12, N], BF16, tag="qkT")  # q pairs 0-5, k 6-11
            vp = cpool.tile([P, NT, H * VW], BF16, tag="vp")  # [v_h | 1] per head
            outT = cpool.tile([P, PAIRS, N], BF16, tag="outT")

            # ---- phase 0: load + cast + transpose ----------------------
            # SWDGE (gpsimd) queue order matters: x casts first so the xT
            # transposes (and with them the first qk groups) start ASAP;
            # w_qkv per-ct so accumulation can begin before the full load;
            # w_proj/bias last (first needed at chunk 1).
            nc.vector.memset(vp[:, :, HD :: VW], 1.0)  # ones cols (denominator)
            # All DMA transfers serialize on one device in the cost model,
            # so ordering is everything: cast x in dual-ct chunks (512B runs
            # avoid the small-element penalty), interleave per-ct transposes
            # and the q/k half of w_qkv (first needed); the v half of w_qkv,
            # w_proj and the bias follow.
            x_bf = dpool.tile([N, C], BF16)
            for cp in range(3):
                csl2 = slice(cp * 2 * P, (cp + 1) * 2 * P)
                nc.gpsimd.dma_start(out=x_bf[:, csl2], in_=x[:, csl2])
                for ct in (2 * cp, 2 * cp + 1):
                    csl = slice(ct * P, (ct + 1) * P)
                    nc.sync.dma_start_transpose(xT[:, ct, :], x_bf[:, csl])
                    nc.gpsimd.dma_start(
                        out=w_qkv_sb[:, ct, 0 : 2 * C],
                        in_=w_qkv[ct * P : (ct + 1) * P, 0 : 2 * C],
                    )
            for ct in range(CT):
                nc.gpsimd.dma_start(
                    out=w_qkv_sb[:, ct, 2 * C : 3 * C],
                    in_=w_qkv[ct * P : (ct + 1) * P, 2 * C : 3 * C],
                )
            nc.gpsimd.dma_start(
                out=wproj_sb[:], in_=w_proj.rearrange("(o p) j -> p o j", p=P)
            )
            nc.sync.dma_start(
                out=b_bc[:], in_=b_proj[None, :].to_broadcast((P, C))
            )

            # ---- emit helpers ------------------------------------------
            def emit_qk_group(jt: int, c4: int):
                """qkT[:, jt, c4*512:(c4+1)*512] = (w_qkv col block)^T x^T.
                psum from the proj pool so the scores double-buffer rhythm
                in the sc pool is never broken mid-chunk."""
                ps = psum_proj.tile([P, 512], F32, tag="proj")
                for ct in range(CT):
                    nc.tensor.matmul(
                        ps[:, 0:QW],
                        lhsT=w_qkv_sb[:, ct, jt * P : (jt + 1) * P],
                        rhs=xT[:, ct, c4 * QW : (c4 + 1) * QW],
                        start=(ct == 0),
                        stop=(ct == CT - 1),
                    )
                # eviction on ACT (Pool cannot read PSUM on TRN2): keeps
                # DVE free for exp tiles
                nc.scalar.copy(
                    out=qkT[:, jt, c4 * QW : (c4 + 1) * QW], in_=ps[:, 0:QW]
                )

            def emit_v_group(nt: int, p: int):
                """vp[:, nt, pair-p head cols] = x-tile @ w_v (natural layout)."""
                ps = psum_proj.tile([P, 512], F32, tag="proj")
                for ct in range(CT):
                    nc.tensor.matmul(
                        ps[:, 0:P],
                        lhsT=xT[:, ct, nt * P : (nt + 1) * P],
                        rhs=w_qkv_sb[:, ct, 2 * C + p * P : 2 * C + (p + 1) * P],
                        start=(ct == 0),
                        stop=(ct == CT - 1),
                    )
                # scatter the two heads' 64-col halves into the 65-col slots
                # (alternate DVE/ACT so neither becomes the chunk-0 choke)
                dst = vp[:, nt, 2 * p * VW : 2 * p * VW + 2 * VW].rearrange(
                    "p (h w) -> p h w", h=2
                )[:, :, 0:HD]
                src = ps[:, 0:P].rearrange("p (h w) -> p h w", h=2)
                if nt % 2 == 0 or os.environ.get("NO_VE_SPLIT", "0") == "1":
                    nc.vector.tensor_copy(out=dst, in_=src)
                else:
                    _activation_on(nc, nc.scalar, dst, src, AF.Copy)

            def emit_proj_group(nt: int, eo: int, ew: int, alt_pool=False):
                """final[nt-tile, eo:eo+ew] = outT^T w_proj + b."""
                if alt_pool:
                    ps = psum_sc.tile([P, 1024], F32, tag="sc")
                else:
                    ps = psum_proj.tile([P, 512], F32, tag="proj")
                for ct in range(CT):
                    nc.tensor.matmul(
                        ps[:, 0:ew],
                        lhsT=outT[:, ct, nt * P : (nt + 1) * P],
                        rhs=wproj_sb[:, ct, eo : eo + ew],
                        start=(ct == 0),
                        stop=(ct == CT - 1),
                    )
                fs = fin_pool.tile([P, 512], F32, tag="fin")
                nc.vector.tensor_tensor(
                    fs[:, 0:ew], ps[:, 0:ew], b_bc[:, eo : eo + ew],
                    mybir.AluOpType.add,
                )
                nc.sync.dma_start(
                    out=out[nt * P : (nt + 1) * P, eo : eo + ew], in_=fs[:, 0:ew]
                )

            def emit_proj_slot(c_done: int, slot: int, alt_pool=False):
                nt = c_done * 4 + slot // 2
                eo, ew = ((0, 512), (512, 256))[slot % 2]
                emit_proj_group(nt, eo, ew, alt_pool)

            in_chunk0 = [True]
            # ---- phase 1 upfront: pair-0 kT + qT(0, chunk0) -------------
            for c4 in range(NCHUNK):
                emit_qk_group(6, c4)
            emit_qk_group(0, 0)

            # ---- phase 2: flat software-pipelined attention stream ------
            # per (c, p): m-sweep over 16 key tiles; attnV lags scores by
            # one step so the PE never waits on the exp engines.
            kt_slots = {1: 0, 4: 1, 7: 2, 10: 3}  # m -> c4 of kT(p+1)
            proj_slots = {  # (p, m) -> slot
                (1, 3): 0, (1, 11): 1, (2, 3): 2, (2, 11): 3,
                (3, 3): 4, (3, 11): 5, (4, 3): 6, (4, 11): 7,
            }

            state = {"i": 0}  # exp tile counter for engine assignment

            def emit_scores(c, p, m):
                qsl = slice(c * QW, (c + 1) * QW)
                msl = slice(m * P, (m + 1) * P)
                sc = psum_sc.tile([P, 1024], F32, tag="sc")
                nc.tensor.matmul(
                    sc[:, 0:QW],
                    lhsT=qkT[0:64, 6 + p, msl],
                    rhs=qkT[0:64, p, qsl],
                    start=True,
                    stop=True,
                )
                nc.tensor.matmul(
                    sc[:, QW : 2 * QW],
                    lhsT=qkT[64:128, 6 + p, msl],
                    rhs=qkT[64:128, p, qsl],
                    start=True,
                    stop=True,
                )
                at = at_pool.tile([P, 1024], BF16, tag="at")
                i = state["i"]
                state["i"] = i + 1
                mode = os.environ.get("EXP_MODE", "mod3")
                if mode == "mod3":
                    if c == 0 or (c == NCHUNK - 1 and p == PAIRS - 1):
                        on_dve = (m % 2 == 1)
                    else:
                        on_dve = (i % 3 == 2)
                    if on_dve:
                        nc.vector.tensor_scalar(
                            out=at[:].bitcast(I16), in0=sc[:],
                            scalar1=EXP_A, scalar2=EXP_B,
                            op0=mybir.AluOpType.mult, op1=mybir.AluOpType.add,
                        )
                    else:
                        _activation_on(nc, nc.scalar, at[:], sc[:], AF.Exp, scale=SCALE)
                elif mode == "mod2":
                    if i % 2 == 1:
                        nc.vector.tensor_scalar(
                            out=at[:].bitcast(I16), in0=sc[:],
                            scalar1=EXP_A, scalar2=EXP_B,
                            op0=mybir.AluOpType.mult, op1=mybir.AluOpType.add,
                        )
                    else:
                        _activation_on(nc, nc.scalar, at[:], sc[:], AF.Exp, scale=SCALE)
                else:  # split768: ACT tiles are 768/256 ACT/DVE; every 3rd all-DVE
                    if i % 3 == 2:
                        nc.vector.tensor_scalar(
                            out=at[:].bitcast(I16), in0=sc[:],
                            scalar1=EXP_A, scalar2=EXP_B,
                            op0=mybir.AluOpType.mult, op1=mybir.AluOpType.add,
                        )
                    else:
                        asl, dsl = (slice(0, 768), slice(768, 1024)) if m % 2 == 0 else (slice(256, 1024), slice(0, 256))
                        _activation_on(nc, nc.scalar, at[:, asl], sc[:, asl], AF.Exp, scale=SCALE)
                        nc.vector.tensor_scalar(
                            out=at[:, dsl].bitcast(I16), in0=sc[:, dsl],
                            scalar1=EXP_A, scalar2=EXP_B,
                            op0=mybir.AluOpType.mult, op1=mybir.AluOpType.add,
                        )
                return at

            def emit_attnv(av_banks, at, p, m):
                for hh in range(2):
                    h = 2 * p + hh
                    for g in range(4):
                        # start zeroes the whole 2KB zero region (the bank),
                        # so only the first col-group starts and only the
                        # last one stops the accumulation group
                        nc.tensor.matmul(
                            av_banks[hh][:, g * VW : (g + 1) * VW],
                            lhsT=at[:, hh * QW + g * P : hh * QW + (g + 1) * P],
                            rhs=vp[:, m, h * VW : (h + 1) * VW],
                            start=(m == 0 and g == 0),
                            stop=(m == NT - 1 and g == 3),
                        )

            def emit_evict(av_banks, c, p):
                """normalize by the accumulated denominators and store outQ,
                then bounce to DRAM and DMA-transpose into outT."""
                rc = recip_pool.tile([P, 8], F32, tag="recip")
                for hh in range(2):
                    nc.vector.reciprocal(
                        rc[:, 4 * hh : 4 * hh + 4],
                        av_banks[hh][:, HD : 4 * VW : VW],
                    )
                oq = oq_pool.tile([P, 4, P], BF16, tag="oq")
                # one normalize instruction per bank: (av * 1.0) * recip
                # with the per-group reciprocal broadcast along the feature
                # dim via a stride-0 AP; frees the psum bank in ~0.5us
                for hh in range(2):
                    rbc = rc[:, 4 * hh : 4 * hh + 4, None].to_broadcast((P, 4, HD))
                    nc.vector.scalar_tensor_tensor(
                        out=oq[:, :, hh * HD : (hh + 1) * HD],
                        in0=av_banks[hh][:, 0 : 4 * VW].rearrange(
                            "p (g w) -> p g w", w=VW
                        )[:, :, 0:HD],
                        scalar=1.0,
                        in1=rbc,
                        op0=mybir.AluOpType.mult,
                        op1=mybir.AluOpType.mult,
                    )
                oqd = oqd_pool.tile([QW, P], BF16)
                nc.sync.dma_start(
                    out=oqd.rearrange("(a p) f -> p a f", p=P), in_=oq[:]
                )
                nc.sync.dma_start_transpose(
                    outT[:, p, c * QW : (c + 1) * QW], oqd[:]
                )

            steps = [
                (c, p, m)
                for c in range(NCHUNK)
                for p in range(PAIRS)
                for m in range(NT)
            ]
            # attnV trails scores by LAG m-steps so the exp engines have
            # ~LAG full steps of latency headroom before the PE needs `at`
            LAG = int(os.environ.get('KLAG', '7'))
            pending = []  # (av_banks, at, c, p, m)
            av_banks = None

            def retire_one():
                pav, pat, pc, pp, pm = pending.pop(0)
                emit_attnv(pav, pat, pp, pm)
                if pm == NT - 1:
                    emit_evict(pav, pc, pp)

            for (c, p, m) in steps:
                if m == 0:
                    av_banks = [
                        psum_av.tile([P, 512], F32, tag="av", name=f"av{c}_{p}_{hh}")
                        for hh in range(2)
                    ]
                # ---- scores + exp for this step ----
                at = emit_scores(c, p, m)
                pending.append((av_banks, at, c, p, m))
                if len(pending) > LAG:
                    retire_one()
                # ---- JIT slots (after the latency-critical work) ----
                in_chunk0[0] = (c == 0)
                if c == 0:
                    emit_v_group(m, p)
                    if p < PAIRS - 1:
                        if m in kt_slots:
                            emit_qk_group(6 + p + 1, kt_slots[m])
                        elif m == 13:
                            emit_qk_group(p + 1, 0)
                if m == 8 and c < NCHUNK - 1:
                    emit_qk_group(p, c + 1)  # next chunk's qT for this pair
                if c > 0 and (p, m) in proj_slots:
                    emit_proj_slot(c - 1, proj_slots[(p, m)])
            while pending:
                retire_one()
            # tail: proj for the last chunk, ping-ponged across two psum
            # pools so consecutive groups double-buffer (sc pool is free now)
            for slot in range(8):
                emit_proj_slot(NCHUNK - 1, slot, alt_pool=(slot % 2 == 1))

    nc.compile()
    return nc


_NC_CACHE: list = []


def _get_nc() -> bass.Bass:
    if not _NC_CACHE:
        _NC_CACHE.append(build_nc())
    return _NC_CACHE[0]


def run(inputs: dict, trace: bool = False):
    """Run on 8 NeuronCores.  Returns (out [B,N,C] f32, exec_time_ns|None)."""
    nc = _get_nc()
    x = np.ascontiguousarray(np.asarray(inputs["x"], dtype=np.float32))
    w_qkv = np.ascontiguousarray(np.asarray(inputs["w_qkv"], dtype=np.float32))
    w_proj = np.ascontiguousarray(np.asarray(inputs["w_proj"], dtype=np.float32))
    b_proj = np.ascontiguousarray(np.asarray(inputs["b_proj"], dtype=np.float32))
    in_maps = [
        {"x": x[i], "w_qkv": w_qkv, "w_proj": w_proj, "b_proj": b_proj}
        for i in range(B)
    ]
    try:
        res = bass_utils.run_bass_kernel_spmd(
            nc, in_maps, core_ids=list(range(B)), trace=trace
        )
    except ModuleNotFoundError:
        res = bass_utils.run_bass_kernel_spmd(
            nc, in_maps, core_ids=list(range(B)), trace=False
        )
    out = np.stack([res.results[i]["out"] for i in range(B)], axis=0)
    return out.astype(np.float32), res.exec_time_ns


def kernel(x, w_qkv, w_proj, b_proj):
    trace = os.environ.get("BASS_KERNEL_TRACE", "0") == "1"
    out, _ = run(
        {"x": x, "w_qkv": w_qkv, "w_proj": w_proj, "b_proj": b_proj}, trace=trace
    )
    return out


# revision 36
# speedup vs baseline: 1.1093x; 1.0027x over previous
"""Trainium2 Bass kernel for multi-head attention (dense transformer block).

Reference computation (per batch element):
    qkv = x @ w_qkv                      # [N, 3C]
    q, k, v = split heads (H=12, HD=64)
    out = softmax(q k^T * HD**-0.5) v    # full [N, N] scores
    out = merge_heads(out) @ w_proj + b_proj

Distribution: pure data parallel over the batch dim — B=8 batch elements,
8 NeuronCores, one element per core.  Weights are replicated.  No
collectives; each core computes its full [2048, 768] output.

Per-core design (cost-model-driven; matmul cost = out-free-size rows):
  * xT [768, 2048] bf16 via cast DMA + DMA-xbar transpose; all phase-0
    DMAs are need-ordered (DMA transfers serialize on one device).
  * qkT[j, n]: q/k for a head pair packed on 128 partitions (head A rows
    0-63, head B rows 64-127).
  * scoresT per (chunk c of 512 queries, pair, key-tile m): keys on
    partitions, queries free.  [128, 1024] (2 heads x 512 q) per m.
  * exp: 2/3 of tiles run exact Exp on ScalarE; every 3rd tile runs a
    Schraudolph approximation on the Vector engine (one tensor_scalar:
    bitcast_bf16(int16(s*A + B)) ~ exp(s/8), ~1.8% rms), keeping either
    engine's throughput and latency below the PE's m-step time.
  * attnV uses `at` as the STATIONARY side: lhsT = at[:, 128-query
    slice] (M=128), rhs = [v_h | ones] [128, 65] -> out [128 q, 65]
    where col 64 accumulates the softmax denominator.  8 matmuls of
    N=65 per m-step (8x65=520 rows vs 2048 in the v-stationary form,
    and the ones column makes the separate denominator matmuls free).
  * per (c, pair): two 1-bank PSUM accumulators (head A/B), 4 query
    groups x 65 cols each; after the 16-m sweep: DVE reciprocal of the
    D columns, then one scalar_tensor_tensor per bank (stride-0
    broadcast recip) evicts normalized [q, feat] bf16 tiles.
  * outQ [q, feat] bounced to DRAM and DMA-xbar transposed into
    outT [768, 2048] (feature-on-partition) for the projection.
  * projection + bias (DVE add): proj for chunk c-1 is slotted through
    chunk c's m-stream; qT prefetch groups borrow the av pool's spare
    psum bank so the scores double-buffer parity is never broken.
  * the whole attention is one flat software-pipelined stream: attnV
    trails scores by LAG=7 m-steps so PE never waits on the exp engines.
"""

import os

import numpy as np

import concourse.bass as bass
import concourse.mybir as mybir
from concourse import bacc, bass_utils
from concourse.tile import TileContext

F32 = mybir.dt.float32
BF16 = mybir.dt.bfloat16
AF = mybir.ActivationFunctionType
IMM = mybir.ImmediateValue

B, N, C = 8, 2048, 768
H, HD = 12, 64
SCALE = HD ** -0.5  # folded into the exp activation
P = 128
NT = N // P          # 16 key tiles
CT = C // P          # 6 feature tiles
NCHUNK = 4           # query chunks of 512
QW = N // NCHUNK     # 512
PAIRS = 6            # head pairs
VW = HD + 1          # 65: v columns + ones column (denominator)

# Schraudolph exp-on-DVE constants: bitcast_bf16(int16(s*EXP_A + EXP_B))
# ~= exp(s*SCALE) with ~1.8% rms multiplicative error.  The int16 convert
# truncates; EXP_C (tuned offline) absorbs that and centres the
# mantissa-interpolation ripple.
LOG2E = 1.4426950408889634
EXP_A = SCALE * LOG2E * 128.0
EXP_C = 6.75
EXP_B = 127.0 * 128.0 - EXP_C
I16 = mybir.dt.int16
# every DVE_EXP_MOD-th exp tile runs on DVE (0 disables)
DVE_EXP_MOD = 3


def _activation_on(nc, eng, out, in_, func, bias=0.0, scale=1.0):
    """InstActivation emitted on an arbitrary engine (ACT or Pool)."""
    if isinstance(bias, float) and func not in (AF.Copy, AF.Reciprocal):
        bias = nc.const_aps.scalar_like(bias, in_)
    ins = [eng.lower_ap(in_)]
    for arg in (bias, scale, 0.0):
        if isinstance(arg, bass.AP):
            ins.append(eng.lower_ap(arg))
        else:
            ins.append(IMM(dtype=mybir.dt.float32, value=arg))
    return eng.add_instruction(
        mybir.InstActivation(
            name=nc.get_next_instruction_name(),
            func=func,
            ins=ins,
            outs=[eng.lower_ap(out)],
        )
    )


def build_nc() -> bass.Bass:
    nc = bacc.Bacc(None)
    x = nc.declare_dram_parameter("x", [N, C], F32, isOutput=False)
    w_qkv = nc.declare_dram_parameter("w_qkv", [C, 3 * C], F32, isOutput=False)
    w_proj = nc.declare_dram_parameter("w_proj", [C, C], F32, isOutput=False)
    b_proj = nc.declare_dram_parameter("b_proj", [C], F32, isOutput=False)
    out = nc.declare_dram_parameter("out", [N, C], F32, isOutput=True)

    with TileContext(nc) as tc:
        with (
            tc.tile_pool(name="const", bufs=1) as cpool,
            tc.tile_pool(name="dram", bufs=1, space="DRAM") as dpool,
            tc.tile_pool(name="oqdram", bufs=2, space="DRAM") as oqd_pool,
            tc.tile_pool(name="at", bufs=9) as at_pool,
            tc.tile_pool(name="oq", bufs=2) as oq_pool,
            tc.tile_pool(name="recip", bufs=2) as recip_pool,
            tc.tile_pool(name="fin", bufs=2) as fin_pool,
            tc.tile_pool(name="psc", bufs=2, space="PSUM") as psum_sc,
            tc.tile_pool(name="pav", bufs=3, space="PSUM") as psum_av,
            tc.tile_pool(name="pproj", bufs=1, space="PSUM") as psum_proj,
        ):
            # ---- persistent SBUF tensors -------------------------------
            w_qkv_sb = cpool.tile([P, CT, 3 * C], BF16, tag="wqkv")
            wproj_sb = cpool.tile([P, CT, C], BF16, tag="wproj")
            b_bc = cpool.tile([P, C], F32, tag="bias")  # bias bcast to 128 rows
            xT = cpool.tile([P, CT, N], BF16, tag="xT")
            qkT = cpool.tile([P, 12, N], BF16, tag="qkT")  # q pairs 0-5, k 6-11
            vp = cpool.tile([P, NT, H * VW], BF16, tag="vp")  # [v_h | 1] per head
            outT = cpool.tile([P, PAIRS, N], BF16, tag="outT")

            # ---- phase 0: load + cast + transpose ----------------------
            # SWDGE (gpsimd) queue order matters: x casts first so the xT
            # transposes (and with them the first qk groups) start ASAP;
            # w_qkv per-ct so accumulation can begin before the full load;
            # w_proj/bias last (first needed at chunk 1).
            nc.vector.memset(vp[:, :, HD :: VW], 1.0)  # ones cols (denominator)
            # All DMA transfers serialize on one device in the cost model,
            # so ordering is everything: cast x in dual-ct chunks (512B runs
            # avoid the small-element penalty), interleave per-ct transposes
            # and the q/k half of w_qkv (first needed); the v half of w_qkv,
            # w_proj and the bias follow.
            x_bf = dpool.tile([N, C], BF16)
            for cp in range(3):
                csl2 = slice(cp * 2 * P, (cp + 1) * 2 * P)
                nc.gpsimd.dma_start(out=x_bf[:, csl2], in_=x[:, csl2])
                for ct in (2 * cp, 2 * cp + 1):
                    csl = slice(ct * P, (ct + 1) * P)
                    nc.sync.dma_start_transpose(xT[:, ct, :], x_bf[:, csl])
                    nc.gpsimd.dma_start(
                        out=w_qkv_sb[:, ct, 0 : 2 * C],
                        in_=w_qkv[ct * P : (ct + 1) * P, 0 : 2 * C],
                    )
            for ct in range(CT):
                nc.gpsimd.dma_start(
                    out=w_qkv_sb[:, ct, 2 * C : 3 * C],
                    in_=w_qkv[ct * P : (ct + 1) * P, 2 * C : 3 * C],
                )
            nc.gpsimd.dma_start(
                out=wproj_sb[:], in_=w_proj.rearrange("(o p) j -> p o j", p=P)
            )
            nc.sync.dma_start(
                out=b_bc[:], in_=b_proj[None, :].to_broadcast((P, C))
            )

            # ---- emit helpers ------------------------------------------
            def emit_qk_group(jt: int, c4: int):
                """qkT[:, jt, c4*512:(c4+1)*512] = (w_qkv col block)^T x^T.
                psum from the proj pool so the scores double-buffer rhythm
                in the sc pool is never broken mid-chunk."""
                ps = psum_proj.tile([P, 512], F32, tag="proj")
                for ct in range(CT):
                    nc.tensor.matmul(
                        ps[:, 0:QW],
                        lhsT=w_qkv_sb[:, ct, jt * P : (jt + 1) * P],
                        rhs=xT[:, ct, c4 * QW : (c4 + 1) * QW],
                        start=(ct == 0),
                        stop=(ct == CT - 1),
                    )
                # eviction on ACT (Pool cannot read PSUM on TRN2): keeps
                # DVE free for exp tiles
                nc.scalar.copy(
                    out=qkT[:, jt, c4 * QW : (c4 + 1) * QW], in_=ps[:, 0:QW]
                )

            def emit_v_group(nt: int, p: int):
                """vp[:, nt, pair-p head cols] = x-tile @ w_v (natural layout)."""
                ps = psum_proj.tile([P, 512], F32, tag="proj")
                for ct in range(CT):
                    nc.tensor.matmul(
                        ps[:, 0:P],
                        lhsT=xT[:, ct, nt * P : (nt + 1) * P],
                        rhs=w_qkv_sb[:, ct, 2 * C + p * P : 2 * C + (p + 1) * P],
                        start=(ct == 0),
                        stop=(ct == CT - 1),
                    )
                # scatter the two heads' 64-col halves into the 65-col slots
                # (alternate DVE/ACT so neither becomes the chunk-0 choke)
                dst = vp[:, nt, 2 * p * VW : 2 * p * VW + 2 * VW].rearrange(
                    "p (h w) -> p h w", h=2
                )[:, :, 0:HD]
                src = ps[:, 0:P].rearrange("p (h w) -> p h w", h=2)
                if nt % 2 == 0 or os.environ.get("NO_VE_SPLIT", "0") == "1":
                    nc.vector.tensor_copy(out=dst, in_=src)
                else:
                    _activation_on(nc, nc.scalar, dst, src, AF.Copy)

            def emit_proj_group(nt: int, eo: int, ew: int, alt_pool=False):
                """final[nt-tile, eo:eo+ew] = outT^T w_proj + b."""
                if alt_pool:
                    ps = psum_sc.tile([P, 1024], F32, tag="sc")
                else:
                    ps = psum_proj.tile([P, 512], F32, tag="proj")
                for ct in range(CT):
                    nc.tensor.matmul(
                        ps[:, 0:ew],
                        lhsT=outT[:, ct, nt * P : (nt + 1) * P],
                        rhs=wproj_sb[:, ct, eo : eo + ew],
                        start=(ct == 0),
                        stop=(ct == CT - 1),
                    )
                fs = fin_pool.tile([P, 512], F32, tag="fin")
                nc.vector.tensor_tensor(
                    fs[:, 0:ew], ps[:, 0:ew], b_bc[:, eo : eo + ew],
                    mybir.AluOpType.add,
                )
                nc.sync.dma_start(
                    out=out[nt * P : (nt + 1) * P, eo : eo + ew], in_=fs[:, 0:ew]
                )

            def emit_proj_slot(c_done: int, slot: int, alt_pool=False):
                nt = c_done * 4 + slot // 2
                eo, ew = ((0, 512), (512, 256))[slot % 2]
                emit_proj_group(nt, eo, ew, alt_pool)

            in_chunk0 = [True]
            # ---- phase 1 upfront: pair-0 kT + qT(0, chunk0) -------------
            for c4 in range(NCHUNK):
                emit_qk_group(6, c4)
            emit_qk_group(0, 0)

            # ---- phase 2: flat software-pipelined attention stream ------
            # per (c, p): m-sweep over 16 key tiles; attnV lags scores by
            # one step so the PE never waits on the exp engines.
            kt_slots = {1: 0, 4: 1, 7: 2, 10: 3}  # m -> c4 of kT(p+1)
            proj_slots = {  # (p, m) -> slot
                (1, 3): 0, (1, 11): 1, (2, 3): 2, (2, 11): 3,
                (3, 3): 4, (3, 11): 5, (4, 3): 6, (4, 11): 7,
            }

            state = {"i": 0}  # exp tile counter for engine assignment

            def emit_scores(c, p, m):
                qsl = slice(c * QW, (c + 1) * QW)
                msl = slice(m * P, (m + 1) * P)
                sc = psum_sc.tile([P, 1024], F32, tag="sc")
                nc.tensor.matmul(
                    sc[:, 0:QW],
                    lhsT=qkT[0:64, 6 + p, msl],
                    rhs=qkT[0:64, p, qsl],
                    start=True,
                    stop=True,
                )
                nc.tensor.matmul(
                    sc[:, QW : 2 * QW],
                    lhsT=qkT[64:128, 6 + p, msl],
                    rhs=qkT[64:128, p, qsl],
                    start=True,
                    stop=True,
                )
                at = at_pool.tile([P, 1024], BF16, tag="at")
                i = state["i"]
                state["i"] = i + 1
                mode = os.environ.get("EXP_MODE", "mod3")
                if mode == "mod3":
                    if c == 0 or (c == NCHUNK - 1 and p == PAIRS - 1):
                        on_dve = (m % 2 == 1)
                    else:
                        on_dve = (i % 3 == 2)
                    if on_dve:
                        nc.vector.tensor_scalar(
                            out=at[:].bitcast(I16), in0=sc[:],
                            scalar1=EXP_A, scalar2=EXP_B,
                            op0=mybir.AluOpType.mult, op1=mybir.AluOpType.add,
                        )
                    else:
                        _activation_on(nc, nc.scalar, at[:], sc[:], AF.Exp, scale=SCALE)
                elif mode == "mod2":
                    if i % 2 == 1:
                        nc.vector.tensor_scalar(
                            out=at[:].bitcast(I16), in0=sc[:],
                            scalar1=EXP_A, scalar2=EXP_B,
                            op0=mybir.AluOpType.mult, op1=mybir.AluOpType.add,
                        )
                    else:
                        _activation_on(nc, nc.scalar, at[:], sc[:], AF.Exp, scale=SCALE)
                else:  # split768: ACT tiles are 768/256 ACT/DVE; every 3rd all-DVE
                    if i % 3 == 2:
                        nc.vector.tensor_scalar(
                            out=at[:].bitcast(I16), in0=sc[:],
                            scalar1=EXP_A, scalar2=EXP_B,
                            op0=mybir.AluOpType.mult, op1=mybir.AluOpType.add,
                        )
                    else:
                        asl, dsl = (slice(0, 768), slice(768, 1024)) if m % 2 == 0 else (slice(256, 1024), slice(0, 256))
                        _activation_on(nc, nc.scalar, at[:, asl], sc[:, asl], AF.Exp, scale=SCALE)
                        nc.vector.tensor_scalar(
                            out=at[:, dsl].bitcast(I16), in0=sc[:, dsl],
                            scalar1=EXP_A, scalar2=EXP_B,
                            op0=mybir.AluOpType.mult, op1=mybir.AluOpType.add,
                        )
                return at

            def emit_attnv(av_banks, at, p, m):
                for hh in range(2):
                    h = 2 * p + hh
                    for g in range(4):
                        # start zeroes the whole 2KB zero region (the bank),
                        # so only the first col-group starts and only the
                        # last one stops the accumulation group
                        nc.tensor.matmul(
                            av_banks[hh][:, g * VW : (g + 1) * VW],
                            lhsT=at[:, hh * QW + g * P : hh * QW + (g + 1) * P],
                            rhs=vp[:, m, h * VW : (h + 1) * VW],
                            start=(m == 0 and g == 0),
                            stop=(m == NT - 1 and g == 3),
                        )

            def emit_evict(av_banks, c, p):
                """normalize by the accumulated denominators and store outQ,
                then bounce to DRAM and DMA-transpose into outT."""
                rc = recip_pool.tile([P, 8], F32, tag="recip")
                for hh in range(2):
                    nc.vector.reciprocal(
                        rc[:, 4 * hh : 4 * hh + 4],
                        av_banks[hh][:, HD : 4 * VW : VW],
                    )
                oq = oq_pool.tile([P, 4, P], BF16, tag="oq")
                # one normalize instruction per bank: (av * 1.0) * recip
                # with the per-group reciprocal broadcast along the feature
                # dim via a stride-0 AP; frees the psum bank in ~0.5us
                for hh in range(2):
                    rbc = rc[:, 4 * hh : 4 * hh + 4, None].to_broadcast((P, 4, HD))
                    nc.vector.scalar_tensor_tensor(
                        out=oq[:, :, hh * HD : (hh + 1) * HD],
                        in0=av_banks[hh][:, 0 : 4 * VW].rearrange(
                            "p (g w) -> p g w", w=VW
                        )[:, :, 0:HD],
                        scalar=1.0,
                        in1=rbc,
                        op0=mybir.AluOpType.mult,
                        op1=mybir.AluOpType.mult,
                    )
                oqd = oqd_pool.tile([QW, P], BF16)
                nc.sync.dma_start(
                    out=oqd.rearrange("(a p) f -> p a f", p=P), in_=oq[:]
                )
                nc.sync.dma_start_transpose(
                    outT[:, p, c * QW : (c + 1) * QW], oqd[:]
                )

            steps = [
                (c, p, m)
                for c in range(NCHUNK)
                for p in range(PAIRS)
                for m in range(NT)
            ]
            # attnV trails scores by LAG m-steps so the exp engines have
            # ~LAG full steps of latency headroom before the PE needs `at`
            LAG = int(os.environ.get('KLAG', '8'))
            pending = []  # (av_banks, at, c, p, m)
            av_banks = None

            def retire_one():
                pav, pat, pc, pp, pm = pending.pop(0)
                emit_attnv(pav, pat, pp, pm)
                if pm == NT - 1:
                    emit_evict(pav, pc, pp)

            for (c, p, m) in steps:
                if m == 0:
                    av_banks = [
                        psum_av.tile([P, 512], F32, tag="av", name=f"av{c}_{p}_{hh}")
                        for hh in range(2)
                    ]
                # ---- scores + exp for this step ----
                at = emit_scores(c, p, m)
                pending.append((av_banks, at, c, p, m))
                if len(pending) > LAG:
                    retire_one()
                # ---- JIT slots (after the latency-critical work) ----
                in_chunk0[0] = (c == 0)
                if c == 0:
                    emit_v_group(m, p)
                    if p < PAIRS - 1:
                        if m in kt_slots:
                            emit_qk_group(6 + p + 1, kt_slots[m])
                        elif m == 13:
                            emit_qk_group(p + 1, 0)
                if m == 8 and c < NCHUNK - 1:
                    emit_qk_group(p, c + 1)  # next chunk's qT for this pair
                if c > 0 and (p, m) in proj_slots:
                    emit_proj_slot(c - 1, proj_slots[(p, m)])
            while pending:
                retire_one()
            # tail: proj for the last chunk, ping-ponged across two psum
            # pools so consecutive groups double-buffer (sc pool is free now)
            for slot in range(8):
                emit_proj_slot(NCHUNK - 1, slot, alt_pool=(slot % 2 == 1))

    nc.compile()
    return nc


_NC_CACHE: list = []


def _get_nc() -> bass.Bass:
    if not _NC_CACHE:
        _NC_CACHE.append(build_nc())
    return _NC_CACHE[0]


def run(inputs: dict, trace: bool = False):
    """Run on 8 NeuronCores.  Returns (out [B,N,C] f32, exec_time_ns|None)."""
    nc = _get_nc()
    x = np.ascontiguousarray(np.asarray(inputs["x"], dtype=np.float32))
    w_qkv = np.ascontiguousarray(np.asarray(inputs["w_qkv"], dtype=np.float32))
    w_proj = np.ascontiguousarray(np.asarray(inputs["w_proj"], dtype=np.float32))
    b_proj = np.ascontiguousarray(np.asarray(inputs["b_proj"], dtype=np.float32))
    in_maps = [
        {"x": x[i], "w_qkv": w_qkv, "w_proj": w_proj, "b_proj": b_proj}
        for i in range(B)
    ]
    try:
        res = bass_utils.run_bass_kernel_spmd(
            nc, in_maps, core_ids=list(range(B)), trace=trace
        )
    except ModuleNotFoundError:
        res = bass_utils.run_bass_kernel_spmd(
            nc, in_maps, core_ids=list(range(B)), trace=False
        )
    out = np.stack([res.results[i]["out"] for i in range(B)], axis=0)
    return out.astype(np.float32), res.exec_time_ns


def kernel(x, w_qkv, w_proj, b_proj):
    trace = os.environ.get("BASS_KERNEL_TRACE", "0") == "1"
    out, _ = run(
        {"x": x, "w_qkv": w_qkv, "w_proj": w_proj, "b_proj": b_proj}, trace=trace
    )
    return out
